# revision 1
# baseline (speedup 1.0000x reference)
"""Trainium2 Bass kernel for nn_MoCo_4810363372846 (retrieval_knn).

Computation (see harness reference):
    h    = relu(im_q @ W1 + b1)            [B, 2048]
    q    = (h @ W2 + b2) row-normalized    [B, 128]
    dist = mean_j sqrt((q_i-k_j) invD (q_i-k_j)^T)  over 64 sampled queue cols
    top-63 (excluding the max) rows of dist gate a masked write into
    output[:, 2:4].

Strategy:
  * Data-parallel over the B=16384 rows: 8 NeuronCores x 2048 rows each.
    Weights / invD / sampled-queue constants are replicated.
  * Host pre-quantizes: X -> e4m3 feature-major [128, 16, rows];
    W1*8192 -> e4m3 [n, 128, 16, 128]; W2*8192 -> e4m3.  Both GEMMs run
    as fp8 DoubleRow matmuls (4x the fp22 PE rate), hidden activations
    stored e4m3, and the Mahalanobis tail in fp22 (f32r).  The serial
    normalize/Mahalanobis chain of each 512-row half-chunk is woven
    between the following half-chunk's GEMM groups so the in-order PE
    never stalls on it.  Device output: dist row [1, 2048] per core.
  * The fp8 pipeline perturbs dist by <~2.3e-2 (measured).  On host:
    gather the 8 dist shards, exactly recompute all rows within WINDOW of
    the top-64 threshold (and of the max), with an adaptive widen-and-
    retry guard, stable-argsort, build the row mask, and apply the masked
    write to output columns 2/3.
"""

import functools
import os

import numpy as np

# diagnosis switches (dev only; default off)
NO_WEAVE = os.environ.get("KERNEL_NO_WEAVE") == "1"
BF16_HT = os.environ.get("KERNEL_BF16_HT") == "1"

B, DIM_MLP, DIM, KQ, NUM = 16384, 2048, 128, 16384, 64
NCORES = 8
BL = B // NCORES  # 2048 rows per core
# batch-chunk processed per pipeline pass (KERNEL_MC: dev-only sim experiment)
MC = int(os.environ.get("KERNEL_MC", "1024"))
NCH = BL // MC
NH = 512          # matmul moving-operand free dim (one PSUM bank of fp32)
MH = MC // NH
P = 128
K16 = DIM_MLP // P  # 16 contraction sub-tiles
SW = 8192.0         # host-side W1 quantization scale (|W1|*SW <= 181 < 240)

# dist window (absolute units) around the top-64 / top-1 thresholds whose
# rows get an exact host-side recompute; ~2x the observed max fp8 error.
WINDOW = 4.5e-2


@functools.lru_cache(maxsize=None)
def _build_nc(reps=1, hw_loop=False):
    import concourse.mybir as mybir
    import concourse.tile as tile
    from concourse import bacc

    f32 = mybir.dt.float32
    f32r = mybir.dt.float32r
    bf16 = mybir.dt.bfloat16
    f8 = mybir.dt.float8e4
    u8 = mybir.dt.uint8
    AF = mybir.ActivationFunctionType
    DR = mybir.MatmulPerfMode.DoubleRow

    nc = bacc.Bacc(None, target_bir_lowering=False)

    x8 = nc.declare_dram_parameter("x8", [P, K16, BL], u8, isOutput=False)
    w1 = nc.declare_dram_parameter("w1", [K16, P, K16, P], u8, isOutput=False)
    w2q = nc.declare_dram_parameter("w2q", [P, K16, P], u8, isOutput=False)
    w2h = (
        nc.declare_dram_parameter("w2h", [P, K16, P], bf16, isOutput=False)
        if BF16_HT
        else None
    )
    b1t = nc.declare_dram_parameter("b1t", [P, K16], f32, isOutput=False)
    b2t = nc.declare_dram_parameter("b2t", [P, 1], f32, isOutput=False)
    invd = nc.declare_dram_parameter("invd", [P, P], f32, isOutput=False)
    ct = nc.declare_dram_parameter("ct", [P, NUM], f32, isOutput=False)
    colc = nc.declare_dram_parameter("colc", [P, 3], f32, isOutput=False)
    rowc = nc.declare_dram_parameter("rowc", [1, NH + NUM + P], f32, isOutput=False)
    dist = nc.declare_dram_parameter("dist", [1, BL], f32, isOutput=True)

    with tile.TileContext(nc) as tc:
        with (
            tc.tile_pool(name="const", bufs=1) as constp,
            tc.tile_pool(name="w1p", bufs=1) as w1p,
            tc.tile_pool(name="xin", bufs=2) as xinp,
            tc.tile_pool(name="ht", bufs=2) as htp,
            tc.tile_pool(name="dsb", bufs=2) as dsbp,
            tc.tile_pool(name="ps_h", bufs=4, space="PSUM") as ps_h,
            tc.tile_pool(name="ps_q", bufs=2, space="PSUM") as ps_q,
            tc.tile_pool(name="ps_d", bufs=2, space="PSUM") as ps_d,
        ):
            # allocate const tiles now; their DMAs are emitted AFTER the
            # first chunk's weight/activation DMAs so the PE's critical path
            # (w1n0 + x8[0]) is at the head of the DMA queue.
            colcs = constp.tile([P, 3], f32r)
            c2col = colcs[:NUM, 2:3].bitcast(f32)
            rowcs = constp.tile([1, NH + NUM + P], f32r)
            ones_k = colcs[:, 0:1]
            ones64s = colcs[:NUM, 1:2]
            negh64 = rowcs[:, NH : NH + NUM]
            ones_m32 = rowcs[:, NH + NUM :]
            b1s = constp.tile([P, K16], f32)
            b2s = constp.tile([P, 1], f32)
            invds = constp.tile([P, P], f32r)
            cts = constp.tile([P, NUM], f32r)
            if BF16_HT:
                w2s = constp.tile([P, K16, P], bf16)
            else:
                w2s = constp.tile([P, K16, P], f8)
            dist_sb = constp.tile([1, BL], f32)
            ht_dt = bf16 if BF16_HT else f8
            qt_scale = 1.0 if BF16_HT else 1.0 / SW

            def dma_consts():
                nc.sync.dma_start(b2s, b2t[:])
                nc.sync.dma_start(colcs, colc[:].bitcast(f32r))
                nc.sync.dma_start(rowcs, rowc[:].bitcast(f32r))
                nc.sync.dma_start(invds, invd[:].bitcast(f32r))
                nc.sync.dma_start(cts, ct[:].bitcast(f32r))
                if BF16_HT:
                    nc.sync.dma_start(w2s, w2h[:])
                else:
                    nc.sync.dma_start(w2s, w2q[:].bitcast(f8))

            # Deferred Mahalanobis-chain steps: each chunk's C/D phase is cut
            # into small steps that get woven between the NEXT chunk's B-phase
            # matmul groups, so the in-order PE never stalls on the serial
            # ACT/DVE chain.
            pending = []

            def emit_one():
                if pending:
                    pending.pop(0)()

            def cd_steps_m(c, m, htc):
                steps = []
                if True:
                    st = {}

                    def s1(m=m, st=st, htc=htc):
                        pq = ps_q.tile([P, NH], f32, tag="pq")
                        if BF16_HT:
                            for k in range(K16):
                                nc.tensor.matmul(
                                    pq,
                                    w2s[:, k, :],
                                    htc[:, k, m * NH : (m + 1) * NH],
                                    start=(k == 0),
                                    stop=(k == K16 - 1),
                                )
                        else:
                            for kk in range(K16 // 2):
                                nc.tensor.matmul(
                                    pq,
                                    w2s[:, 2 * kk : 2 * kk + 2, :],
                                    htc[:, 2 * kk : 2 * kk + 2, m * NH : (m + 1) * NH],
                                    start=(kk == 0),
                                    stop=(kk == K16 // 2 - 1),
                                    perf_mode=DR,
                                )
                        qt = dsbp.tile([P, NH], f32, tag="qt")
                        nc.scalar.activation(
                            qt, pq, AF.Identity, bias=b2s[:, 0:1], scale=qt_scale
                        )
                        st["qt"] = qt

                    def s2(st=st):
                        qt = st["qt"]
                        sq = dsbp.tile([P, NH], f32r, tag="sq")
                        nc.vector.tensor_mul(sq, qt, qt)
                        pn = ps_d.tile([P, NH], f32, tag="pd")
                        nc.tensor.matmul(pn[:1, :], ones_k, sq)
                        st["pn"] = pn

                    def s3(st=st):
                        nrm = dsbp.tile([1, NH], f32, tag="nrm")
                        nc.scalar.activation(nrm, st["pn"][:1, :], AF.Sqrt)
                        s = dsbp.tile([1, NH], f32r, tag="s")
                        with nc.allow_low_precision("f32r==fp32 on DVE"):
                            nc.vector.reciprocal(s, nrm)
                        pb = ps_d.tile([P, NH], f32, tag="pd")
                        nc.tensor.matmul(pb, ones_m32, s)
                        st["pb"] = pb

                    def s4(st=st):
                        qn = dsbp.tile([P, NH], f32r, tag="qn")
                        nc.vector.tensor_mul(qn, st["qt"], st["pb"])
                        pu = ps_d.tile([P, NH], f32, tag="pd")
                        nc.tensor.matmul(pu, invds, qn)
                        st["qn"] = qn
                        st["pu"] = pu

                    def s5(st=st):
                        prod = dsbp.tile([P, NH], f32r, tag="prod")
                        nc.vector.tensor_mul(prod, st["qn"], st["pu"])
                        pr = ps_d.tile([P, NH], f32, tag="pd")
                        nc.tensor.matmul(pr[:1, :], ones_k, prod)
                        rsb = dsbp.tile([1, NH], f32r, tag="rsb")
                        nc.scalar.activation(rsb, pr[:1, :], AF.Identity)
                        st["rsb"] = rsb

                    def s6(st=st):
                        # psum = t - r/2 ; quad = -2*psum + c2 (c2 folded into
                        # the Sqrt activation's per-partition bias)
                        ptq = ps_d.tile([P, NH], f32, tag="pd")
                        nc.tensor.matmul(
                            ptq[:NUM, :], cts, st["qn"], start=True, stop=False
                        )
                        nc.tensor.matmul(
                            ptq[:NUM, :], negh64, st["rsb"], start=False, stop=True
                        )
                        sqq = dsbp.tile([NUM, NH], f32r, tag="sqq")
                        nc.scalar.activation(
                            sqq, ptq[:NUM, :], AF.Sqrt, scale=-2.0, bias=c2col
                        )
                        st["sqq"] = sqq

                    def s7(c=c, m=m, st=st):
                        pdd = ps_d.tile([P, NH], f32, tag="pd")
                        nc.tensor.matmul(pdd[:1, :], ones64s, st["sqq"])
                        o0 = c * MC + m * NH
                        nc.scalar.activation(
                            dist_sb[:, o0 : o0 + NH], pdd[:1, :], AF.Identity
                        )

                    steps += [s1, s2, s3, s4, s5, s6, s7]
                return steps

            def dma_x8(g):
                c = g % NCH
                pair = []
                for m in range(MH):
                    t = xinp.tile([P, K16, NH], f8, tag=f"x8{m}")
                    o0 = c * MC + m * NH
                    nc.sync.dma_start(t, x8.bitcast(f8)[:, :, o0 : o0 + NH])
                    pair.append(t)
                return pair

            G = reps * NCH

            def dma_w1(n):
                t = w1p.tile([P, K16, P], f8, tag=f"w1n{n}", name=f"w1n{n}")
                nc.sync.dma_start(t, w1[n].bitcast(f8))
                return t

            def emit_chunk(c, w1t, x8c):
                # ---- h = relu((X8 @ W8)/SW + b1), stored e4m3 ----
                # m-outer so each m-half's Mahalanobis chain weaves into the
                # next m-half's GEMM groups (halves the end-of-kernel drain).
                htc = htp.tile([P, K16, MC], ht_dt, tag="htc")
                for m in range(MH):
                    for n in range(K16):
                        ph = ps_h.tile([P, NH], f32, tag="ph")
                        for kk in range(K16 // 2):
                            nc.tensor.matmul(
                                ph,
                                w1t[n][:, 2 * kk : 2 * kk + 2, :],
                                x8c[m][:, 2 * kk : 2 * kk + 2, :],
                                start=(kk == 0),
                                stop=(kk == K16 // 2 - 1),
                                perf_mode=DR,
                            )
                        nc.scalar.activation(
                            htc[:, n, m * NH : (m + 1) * NH],
                            ph,
                            AF.Relu,
                            bias=b1s[:, n : n + 1],
                            scale=1.0 / SW,
                        )
                        emit_one()
                    pending.extend(cd_steps_m(c, m, htc))
                if NO_WEAVE:
                    while pending:
                        emit_one()

            # head-critical DMA order: w1n0, first x8 halves, rest of w1,
            # then the small constants (first consumed only ~1 B-group in).
            nc.sync.dma_start(b1s, b1t[:])
            if hw_loop:
                dma_consts()
                with tc.For_i(0, reps, 1):
                    w1t = [dma_w1(n) for n in range(K16)]
                    xs = [dma_x8(c) for c in range(NCH)]
                    for c in range(NCH):
                        emit_chunk(c, w1t, xs[c])
                    while pending:
                        emit_one()
            else:
                w1t = [dma_w1(0)]
                nxt = dma_x8(0)
                w1t += [dma_w1(n) for n in range(1, K16)]
                dma_consts()
                for g in range(G):
                    c = g % NCH
                    x8c = nxt
                    if c == 0 and g > 0:
                        w1t = [dma_w1(n) for n in range(K16)]
                    if g + 1 < G:
                        nxt = dma_x8(g + 1)  # prefetch next chunk
                    emit_chunk(c, w1t, x8c)
                while pending:
                    emit_one()
            nc.sync.dma_start(dist[:], dist_sb)

    nc.compile()
    return nc


def _host_constants(W1, b1, W2, b2, queue, invD, sample_idx):
    import ml_dtypes

    E4 = ml_dtypes.float8_e4m3
    qs = queue[:, sample_idx].T.astype(np.float64)  # [64, 128]
    iD = invD.astype(np.float64)
    ct = (iD @ qs.T).astype(np.float32)  # [128, 64]
    c2 = np.sum((qs @ iD) * qs, axis=1).astype(np.float32)[None, :]  # [1, 64]
    b1t = np.ascontiguousarray(
        b1.astype(np.float32).reshape(K16, P).T
    )  # [128, 16]; b1t[p, no] = b1[no*128+p]
    b2t = np.ascontiguousarray(b2.astype(np.float32).reshape(P, 1))
    colc = np.zeros((P, 3), np.float32)
    colc[:, 0] = 1.0
    colc[:, 1] = 1.0 / NUM
    colc[:NUM, 2] = c2[0]
    rowc = np.zeros((1, NH + NUM + P), np.float32)
    rowc[0, :NH] = -0.5
    rowc[0, NH : NH + NUM] = -0.5
    rowc[0, NH + NUM :] = 1.0
    # w1q[n, p, ko, m] = e4m3(W1[ko*128+p, n*128+m] * SW)
    w1q = np.ascontiguousarray(
        (W1 * np.float32(SW))
        .astype(E4)
        .reshape(K16, P, K16, P)
        .transpose(2, 1, 0, 3)
    ).view(np.uint8)
    # w2q[p, ko, d] = e4m3(W2[ko*128+p, d] * SW)
    w2q = np.ascontiguousarray(
        (W2 * np.float32(SW))
        .astype(E4)
        .reshape(K16, P, DIM)
        .transpose(1, 0, 2)
    ).view(np.uint8)
    # w2h[p, ko, d] = bf16(W2[ko*128+p, d])  (diagnosis variant)
    w2h = np.ascontiguousarray(
        W2.astype(ml_dtypes.bfloat16).reshape(K16, P, DIM).transpose(1, 0, 2)
    )
    return ct, c2, b1t, b2t, w1q, w2q, w2h, colc, rowc


def _host_x8(im_q):
    import ml_dtypes

    # x8[c, p, ko, b] = e4m3(im_q[c*BL + b, ko*128 + p])
    return np.ascontiguousarray(
        im_q.astype(ml_dtypes.float8_e4m3)
        .reshape(NCORES, BL, K16, P)
        .transpose(0, 3, 2, 1)
    ).view(np.uint8)


def per_core_inputs(inp):
    """Build the per-core input maps (host prep). Returns list of dicts."""
    im_q = np.ascontiguousarray(np.asarray(inp["im_q"], dtype=np.float32))
    W1 = np.ascontiguousarray(np.asarray(inp["W1"], dtype=np.float32))
    b1 = np.asarray(inp["b1"], dtype=np.float32)
    W2 = np.ascontiguousarray(np.asarray(inp["W2"], dtype=np.float32))
    b2 = np.asarray(inp["b2"], dtype=np.float32)
    queue = np.asarray(inp["queue"], dtype=np.float32)
    invD = np.ascontiguousarray(np.asarray(inp["invD"], dtype=np.float32))
    sample_idx = np.asarray(inp["sample_idx"])

    ct, c2, b1t, b2t, w1q, w2q, w2h, colc, rowc = _host_constants(
        W1, b1, W2, b2, queue, invD, sample_idx
    )
    x8 = _host_x8(im_q)
    in_maps = []
    for i in range(NCORES):
        in_maps.append(
            {
                "x8": x8[i],
                "w1": w1q,
                "w2q": w2q,
                **({"w2h": w2h} if BF16_HT else {}),
                "b1t": b1t,
                "b2t": b2t,
                "invd": invD,
                "ct": ct,
                "colc": colc,
                "rowc": rowc,
            }
        )
    return in_maps


def _exact_dist_rows(rows, im_q, W1, b1, W2, b2, qs64, iD64):
    X = im_q[rows].astype(np.float32)
    h = np.maximum(
        (X @ W1.astype(np.float32)).astype(np.float64) + b1.astype(np.float64), 0
    )
    q = h @ W2.astype(np.float64) + b2.astype(np.float64)
    q = q / np.maximum(np.linalg.norm(q, axis=1, keepdims=True), 1e-12)
    u = q @ iD64
    r = np.sum(u * q, axis=1)
    t = q @ (iD64 @ qs64.T)
    c2 = np.sum((qs64 @ iD64) * qs64, axis=1)
    quad = np.maximum(r[:, None] + c2[None, :] - 2 * t, 0)
    return np.sqrt(quad).mean(axis=1)


LAST_RESULTS = None  # for test harness introspection
LAST_STATS = None  # recompute-row count + observed fp8 boundary error


def kernel(im_q, output, sample_idx, W1, b1, W2, b2, queue, invD):
    global LAST_RESULTS, LAST_STATS
    from concourse.bass_utils import run_bass_kernel_spmd

    inp = {
        "im_q": im_q, "W1": W1, "b1": b1, "W2": W2, "b2": b2,
        "queue": queue, "invD": invD, "sample_idx": sample_idx,
    }
    im_q = np.ascontiguousarray(np.asarray(im_q, dtype=np.float32))
    output = np.asarray(output, dtype=np.float32)
    W1 = np.ascontiguousarray(np.asarray(W1, dtype=np.float32))
    b1 = np.asarray(b1, dtype=np.float32)
    W2 = np.ascontiguousarray(np.asarray(W2, dtype=np.float32))
    b2 = np.asarray(b2, dtype=np.float32)
    queue = np.asarray(queue, dtype=np.float32)
    invD = np.ascontiguousarray(np.asarray(invD, dtype=np.float32))
    sample_idx = np.asarray(sample_idx)

    in_maps = per_core_inputs(inp)
    nc = _build_nc()
    res = run_bass_kernel_spmd(nc, in_maps, core_ids=list(range(NCORES)))
    LAST_RESULTS = res
    dist = np.concatenate(
        [np.asarray(res.results[i]["dist"]).reshape(BL) for i in range(NCORES)]
    ).astype(np.float64)

    # exact host recompute of rows near the top-64 inclusion boundary (and
    # the max-exclusion boundary) so fp8 error cannot flip the selected set
    qs64 = queue[:, sample_idx].T.astype(np.float64)
    iD64 = invD.astype(np.float64)
    win = WINDOW
    done = np.zeros(B, dtype=bool)
    max_err = 0.0
    for _attempt in range(4):
        thr = np.partition(dist, B - NUM)[B - NUM]
        top1 = dist.max()
        rows = np.nonzero(
            ((np.abs(dist - thr) <= win) | (dist >= top1 - win)) & ~done
        )[0]
        if rows.size:
            prev = dist[rows].copy()
            dist[rows] = _exact_dist_rows(
                rows, im_q, W1, b1, W2, b2, qs64, iD64
            )
            max_err = max(max_err, float(np.abs(dist[rows] - prev).max()))
            done[rows] = True
        # converged when every row within win/2 of the (updated) boundaries
        # has been exactly recomputed
        thr = np.partition(dist, B - NUM)[B - NUM]
        top1 = dist.max()
        chk = np.nonzero(
            ((np.abs(dist - thr) <= win / 2) | (dist >= top1 - win / 2)) & ~done
        )[0]
        if chk.size == 0:
            break

    LAST_STATS = {
        "recompute_rows": int(done.sum()),
        "max_fp8_err_at_boundary": max_err,
        "window": win,
    }
    order = np.argsort(dist, kind="stable")
    sel = order[-NUM:-1]
    row_mask = np.zeros(B, dtype=bool)
    row_mask[sel] = True
    cond = row_mask & ((np.abs(output[:, 2]) < 1.0) | (np.abs(output[:, 3]) < 1.0))
    out = output.copy()
    out[:, 2] = np.where(cond, np.float32(-5.0), output[:, 2])
    out[:, 3] = np.where(cond, np.float32(5.0), out[:, 3])
    return out



# revision 2
# speedup vs baseline: 1542.4932x; 1542.4932x over previous
"""Trainium2 Bass kernel for nn_MoCo_4810363372846 (retrieval_knn) — v2.

Computation (see harness reference):
    h    = relu(im_q @ W1 + b1)            [B, 2048]
    q    = (h @ W2 + b2) row-normalized    [B, 128]
    dist = mean_j sqrt((q_i-k_j) invD (q_i-k_j)^T)  over 64 sampled queue cols
    top-63 (excluding the max) rows of dist gate a masked write into
    output[:, 2:4].

Strategy (v2):
  * Data-parallel over the B=16384 rows: 8 NeuronCores x 2048 rows each.
  * fp8 DoubleRow GEMMs for both layers (4x bf16 PE rate); hidden
    activations e4m3.
  * Row-major Mahalanobis tail: for each 128-row tile, the PE (with q as
    the bf16 stationary operand) emits q^T, z=q^T invD and T=q^T (invD K);
    DVE tensor_tensor_reduce fuses the row reductions (|q|^2, q^T invD q);
    one scalar_tensor_tensor assembles u*T - c2/2 and a single ACT Sqrt
    with per-partition bias + free-dim accumulator emits dist for 128
    rows.  Normalization is folded into per-row scalars (no explicit
    normalize of q).
  * Two-pass GEMM1 schedule (n0-7 for all four 512-row m-halves, then
    n8-15): the first pass only needs half of W1 up front, so the PE
    never waits long on the serialized DMA pipe.
  * Device output: dist [128, 16] per core (dist[p,t] = row t*128+p).
"""

import functools
import os

import numpy as np

B, DIM_MLP, DIM, KQ, NUM = 16384, 2048, 128, 16384, 64
NCORES = 8
P = 128
BL = B // NCORES        # 2048 rows per core
NH = 512                # rows per m-half (PSUM bank of fp32)
MH = BL // NH           # 4 m-halves
NT = NH // P            # 4 row-tiles per m-half
K16 = DIM_MLP // P      # 16 contraction sub-tiles
KA = 8                  # A-phase covers w1 n-tiles [0, KA)
SW = 8192.0             # host-side W1/W2 quantization scale
Q64 = 4096.0            # = NUM^2; folded so ACT accum emits mean directly

# dist window (absolute units) around the top-64 / top-1 thresholds whose
# rows get an exact host-side recompute; ~2.5x the observed max device err.
WINDOW = float(os.environ.get("KERNEL_WINDOW", "0.06"))


@functools.lru_cache(maxsize=None)
def _build_nc(reps=1, hw_loop=False):
    import concourse.mybir as mybir
    import concourse.tile as tile
    from concourse import bacc

    f32 = mybir.dt.float32
    f32r = mybir.dt.float32r
    bf16 = mybir.dt.bfloat16
    f8 = mybir.dt.float8e4
    u8 = mybir.dt.uint8
    AF = mybir.ActivationFunctionType
    ALU = mybir.AluOpType
    DR = mybir.MatmulPerfMode.DoubleRow

    nc = bacc.Bacc(None, target_bir_lowering=False)

    x8 = nc.declare_dram_parameter("x8", [P, K16, BL], u8, isOutput=False)
    w1 = nc.declare_dram_parameter("w1", [K16, P, K16, P], u8, isOutput=False)
    w2q = nc.declare_dram_parameter("w2q", [P, K16, P], u8, isOutput=False)
    b1t = nc.declare_dram_parameter("b1t", [P, K16], f32, isOutput=False)
    b2t = nc.declare_dram_parameter("b2t", [P, 1], f32, isOutput=False)
    invd = nc.declare_dram_parameter("invd", [P, P], bf16, isOutput=False)
    ct = nc.declare_dram_parameter("ct", [P, NUM], bf16, isOutput=False)
    c2b = nc.declare_dram_parameter("c2b", [P, NUM], f32, isOutput=False)
    identt = nc.declare_dram_parameter("identt", [P, P], bf16, isOutput=False)
    dist = nc.declare_dram_parameter("dist", [P, MH * NT], f32, isOutput=True)

    with tile.TileContext(nc) as tc:
        with (
            tc.tile_pool(name="const", bufs=1) as constp,
            tc.tile_pool(name="w1p", bufs=2) as w1p,
            tc.tile_pool(name="xin", bufs=2) as xinp,
            tc.tile_pool(name="ht", bufs=1) as htp,
            tc.tile_pool(name="qtp", bufs=2) as qtp,
            tc.tile_pool(name="sc", bufs=2) as scp,
            tc.tile_pool(name="rows", bufs=4) as rowsp,
            tc.tile_pool(name="ps_h", bufs=3, space="PSUM") as ps_h,
            tc.tile_pool(name="ps_q", bufs=2, space="PSUM") as ps_q,
            tc.tile_pool(name="ps_c", bufs=3, space="PSUM") as ps_c,
        ):
            b1s = constp.tile([P, K16], f32)
            b2s = constp.tile([P, 1], f32)
            invs = constp.tile([P, P], bf16)
            cts = constp.tile([P, NUM], bf16)
            c2s = constp.tile([P, NUM], f32)
            idents = constp.tile([P, P], bf16)
            w2s = constp.tile([P, K16, P], f8)
            dist_sb = constp.tile([P, MH * NT], f32)
            warm = constp.tile([P, 1], f32)

            # ---- deferred chain steps, woven between GEMM1 groups ----
            pending = []

            def emit_one():
                if pending:
                    pending.pop(0)()

            def chain_steps(m, htc, fine_tail=False):
                """Stage-paired bundles for m-half m: GEMM2 + qt, then per
                128-row tile: PE (transpose/z/T), DVE reductions + scalars,
                ACT sqrt-accumulate into a dist column."""
                qt = qtp.tile([P, NH], bf16, tag=f"qt{m % 2}", name="qt")
                sts = [dict() for _ in range(NT)]

                def sA(m=m, htc=htc, qt=qt):
                    pq = ps_q.tile([P, NH], f32, tag="pq")
                    for kk in range(K16 // 2):
                        nc.tensor.matmul(
                            pq,
                            w2s[:, 2 * kk : 2 * kk + 2, :],
                            htc[:, 2 * kk : 2 * kk + 2, :],
                            start=(kk == 0),
                            stop=(kk == K16 // 2 - 1),
                            perf_mode=DR,
                        )
                    nc.scalar.activation(
                        qt, pq, AF.Identity, bias=b2s[:, 0:1], scale=1.0 / SW
                    )

                def sG2(t, m=m, htc=htc, qt=qt):
                    # per-tile GEMM2 + qt slice (tail only)
                    pqt = ps_q.tile([P, NH], f32, tag="pq", name="pqt")
                    pq = pqt[:, 0:P]
                    for kk in range(K16 // 2):
                        nc.tensor.matmul(
                            pq,
                            w2s[:, 2 * kk : 2 * kk + 2, :],
                            htc[:, 2 * kk : 2 * kk + 2, t * P : (t + 1) * P],
                            start=(kk == 0),
                            stop=(kk == K16 // 2 - 1),
                            perf_mode=DR,
                        )
                    nc.scalar.activation(
                        qt[:, t * P : (t + 1) * P],
                        pq,
                        AF.Identity,
                        bias=b2s[:, 0:1],
                        scale=1.0 / SW,
                    )

                def sB(t, qt=qt, sts=sts):
                    # one PSUM bank holds z | T | xp | qtT per tile
                    st = sts[t]
                    qc = qt[:, t * P : (t + 1) * P]
                    cb = ps_c.tile([P, NH], f32, tag="cb", name="cb")
                    z = cb[:, 0:P]
                    T = cb[:, P : P + NUM]
                    qtT = cb[:, P + NUM : P + NUM + P // 2].bitcast(bf16)
                    nc.tensor.transpose(qtT, qc, idents)
                    nc.tensor.matmul(z, qc, invs)
                    nc.tensor.matmul(T, qc, cts)
                    st["qtT"], st["z"], st["T"] = qtT, z, T

                def sD(t, sts=sts):
                    st = sts[t]
                    qtT, z = st["qtT"], st["z"]
                    qtTs = scp.tile([P, P], bf16, tag="qtTs", name="qtTs")
                    nc.vector.tensor_copy(qtTs, qtT)
                    snb = scp.tile([P, P], bf16, tag="snb", name="snb")
                    n2 = rowsp.tile([P, 1], f32, tag="n2", name="n2")
                    nc.vector.scalar_tensor_tensor(
                        snb, qtTs, 1.0, qtTs, ALU.mult, ALU.mult, accum_out=n2
                    )
                    szb = scp.tile([P, P], bf16, tag="szb", name="szb")
                    R = rowsp.tile([P, 1], f32, tag="R", name="R")
                    nc.vector.scalar_tensor_tensor(
                        szb, z, 1.0, qtTs, ALU.mult, ALU.mult, accum_out=R
                    )
                    w = rowsp.tile([P, 1], f32, tag="w", name="w")
                    nc.vector.reciprocal(w, n2)
                    Rw = rowsp.tile([P, 1], f32, tag="Rw", name="Rw")
                    nc.vector.tensor_scalar(Rw, R, w, 1.0 / Q64, ALU.mult, ALU.mult)
                    st["w"], st["Rw"] = w, Rw

                def sE(t, m=m, sts=sts):
                    st = sts[t]
                    u = rowsp.tile([P, 1], f32, tag="u", name="u")
                    nc.scalar.activation(u, st["w"], AF.Sqrt)
                    xp = scp.tile([P, NUM], f32, tag="xp", name="xp")
                    nc.vector.scalar_tensor_tensor(
                        xp, st["T"], u, c2s, ALU.mult, ALU.subtract
                    )
                    sqs = scp.tile([P, NUM], bf16, tag="sqs", name="sqs")
                    g = m * NT + t
                    nc.scalar.activation(
                        sqs,
                        xp,
                        AF.Sqrt,
                        bias=st["Rw"],
                        scale=-2.0 / Q64,
                        accum_out=dist_sb[:, g : g + 1],
                    )

                def sOut(m=m):
                    nc.sync.dma_start(
                        dist[:, m * NT : (m + 1) * NT],
                        dist_sb[:, m * NT : (m + 1) * NT],
                    )

                if not fine_tail:
                    steps = [
                        sA,
                        lambda: sB(0),
                        lambda: sB(1),
                        lambda: sD(0),
                        lambda: (sB(2), sD(1)),
                        lambda: (sE(0), sB(3), sD(2)),
                        lambda: (sE(1), sD(3)),
                        lambda: sE(2),
                        lambda: (sE(3), sOut()),
                    ]
                else:
                    steps = [
                        lambda: sG2(0),
                        lambda: sG2(1),
                        lambda: sG2(2),
                        lambda: (sB(0), sD(0)),
                        lambda: sG2(3),
                        lambda: (sB(1), sD(1), sE(0)),
                        lambda: (sB(2), sD(2), sE(1)),
                        lambda: (sB(3), sD(3), sE(2)),
                        lambda: (sE(3), sOut()),
                    ]
                return steps

            def dma_w1(n):
                t = w1p.tile([P, K16, P], f8, tag=f"w1n{n}", name=f"w1n{n}")
                nc.sync.dma_start(t, w1[n].bitcast(f8))
                return t

            def dma_x8_m(m, pieces=1):
                """Returns (list of piece tiles, k-tiles per piece).  Pieces
                are separate tiles so a consumer of an early k-tile does not
                wait on later pieces (tile-granular dependencies)."""
                o0 = m * NH
                kper = K16 // pieces
                ts = []
                for a in range(pieces):
                    t = xinp.tile(
                        [P, kper, NH], f8, tag=f"x8m{m}p{a}", name=f"x8m{m}p{a}"
                    )
                    nc.sync.dma_start(
                        t,
                        x8.bitcast(f8)[:, a * kper : (a + 1) * kper, o0 : o0 + NH],
                    )
                    ts.append(t)
                return ts, kper

            def dma_consts_g2():
                nc.sync.dma_start(b2s, b2t[:])
                nc.sync.dma_start(w2s, w2q[:].bitcast(f8))

            def dma_consts_chain():
                nc.sync.dma_start(invs, invd[:])
                nc.sync.dma_start(cts, ct[:])
                nc.sync.dma_start(c2s, c2b[:])
                nc.sync.dma_start(idents, identt[:])

            def dma_consts():
                dma_consts_g2()
                dma_consts_chain()

            def emit_group(m, n, w1t, x8m, htc):
                pieces, kper = x8m
                ph = ps_h.tile([P, NH], f32, tag="ph", name="ph")
                for kk in range(K16 // 2):
                    k0 = 2 * kk
                    pc = pieces[k0 // kper]
                    off = k0 % kper
                    nc.tensor.matmul(
                        ph,
                        w1t[n][:, k0 : k0 + 2, :],
                        pc[:, off : off + 2, :],
                        start=(kk == 0),
                        stop=(kk == K16 // 2 - 1),
                        perf_mode=DR,
                    )
                nc.scalar.activation(
                    htc[:, n, :], ph, AF.Relu, bias=b1s[:, n : n + 1], scale=1.0 / SW
                )
                emit_one()

            def emit_rep(w1t, xs, htcs):
                for m in range(MH):
                    for n in range(K16):
                        emit_group(m, n, w1t, xs[m], htcs[m])
                    steps = chain_steps(m, htcs[m], fine_tail=(m == MH - 1))
                    if m == 0:
                        # eager: GEMM2(m0) fills the PE's wait for x8 m1
                        # (still in the DMA pipe)
                        steps[0]()
                        pending.extend(steps[1:])
                    else:
                        pending.extend(steps)

            def alloc_htcs():
                return [
                    htp.tile([P, K16, NH], f8, tag=f"h{m}", name=f"h{m}")
                    for m in range(MH)
                ]

            # ---- head: force the sqrt act table while DMA streams ----
            nc.vector.memset(warm, 1.0)
            nc.scalar.activation(warm, warm, AF.Sqrt)

            if hw_loop:
                nc.sync.dma_start(b1s, b1t[:])
                dma_consts()
                with tc.For_i(0, reps, 1):
                    w1t = [dma_w1(n) for n in range(K16)]
                    xs = [dma_x8_m(m) for m in range(MH)]
                    emit_rep(w1t, xs, alloc_htcs())
                    while pending:
                        emit_one()
            else:
                for r in range(reps):
                    if r == 0:
                        # w1n0, x8 m0 (pieces), b1, w1n1-7, consts (needed
                        # by the eagerly-emitted m0 chain), w1n8-15, x8 m1
                        # (pieces: partial m1 groups can start as they
                        # land), x8 m2-3.  m0 is fed at stream rate and the
                        # x8 m1 wait is filled with chain + partial work.
                        w1t = [dma_w1(0)]
                        xs = [dma_x8_m(0, pieces=4)]
                        nc.sync.dma_start(b1s, b1t[:])
                        w1t += [dma_w1(n) for n in range(1, K16)]
                        dma_consts_g2()
                        xs.append(dma_x8_m(1))
                        dma_consts_chain()
                        xs += [dma_x8_m(m) for m in range(2, MH)]
                    else:
                        w1t = [dma_w1(0)]
                        xs = [dma_x8_m(0)]
                        w1t += [dma_w1(n) for n in range(1, K16)]
                        xs += [dma_x8_m(m) for m in range(1, MH)]
                    emit_rep(w1t, xs, alloc_htcs())
                while pending:
                    emit_one()

    nc.compile()
    return nc


def _host_constants(W1, b1, W2, b2, queue, invD, sample_idx):
    import ml_dtypes

    E4 = ml_dtypes.float8_e4m3
    BF = ml_dtypes.bfloat16
    qs = queue[:, sample_idx].T.astype(np.float64)  # [64, 128]
    iD = invD.astype(np.float64)
    ct = (iD @ qs.T).astype(np.float32)  # [128, 64]
    c2 = np.sum((qs @ iD) * qs, axis=1).astype(np.float32)  # [64]
    b1t = np.ascontiguousarray(b1.astype(np.float32).reshape(K16, P).T)
    b2t = np.ascontiguousarray(b2.astype(np.float32).reshape(P, 1))
    c2b = np.ascontiguousarray(
        np.broadcast_to((c2 / 2.0)[None, :], (P, NUM)).astype(np.float32)
    )
    identt = np.eye(P, dtype=BF)
    w1q = np.ascontiguousarray(
        (W1 * np.float32(SW))
        .astype(E4)
        .reshape(K16, P, K16, P)
        .transpose(2, 1, 0, 3)
    ).view(np.uint8)
    w2q = np.ascontiguousarray(
        (W2 * np.float32(SW)).astype(E4).reshape(K16, P, DIM).transpose(1, 0, 2)
    ).view(np.uint8)
    return ct, c2, b1t, b2t, w1q, w2q, c2b, identt


def _host_x8(im_q):
    import ml_dtypes

    return np.ascontiguousarray(
        im_q.astype(ml_dtypes.float8_e4m3)
        .reshape(NCORES, BL, K16, P)
        .transpose(0, 3, 2, 1)
    ).view(np.uint8)


def per_core_inputs(inp):
    im_q = np.ascontiguousarray(np.asarray(inp["im_q"], dtype=np.float32))
    W1 = np.ascontiguousarray(np.asarray(inp["W1"], dtype=np.float32))
    b1 = np.asarray(inp["b1"], dtype=np.float32)
    W2 = np.ascontiguousarray(np.asarray(inp["W2"], dtype=np.float32))
    b2 = np.asarray(inp["b2"], dtype=np.float32)
    queue = np.asarray(inp["queue"], dtype=np.float32)
    invD = np.ascontiguousarray(np.asarray(inp["invD"], dtype=np.float32))
    sample_idx = np.asarray(inp["sample_idx"])

    ct, c2, b1t, b2t, w1q, w2q, c2b, identt = _host_constants(
        W1, b1, W2, b2, queue, invD, sample_idx
    )
    import ml_dtypes

    BF = ml_dtypes.bfloat16
    invb = np.ascontiguousarray(invD.astype(BF))
    ctb = np.ascontiguousarray(ct.astype(BF))
    x8 = _host_x8(im_q)
    in_maps = []
    for i in range(NCORES):
        in_maps.append(
            {
                "x8": x8[i],
                "w1": w1q,
                "w2q": w2q,
                "b1t": b1t,
                "b2t": b2t,
                "invd": invb,
                "ct": ctb,
                "c2b": c2b,
                "identt": identt,
            }
        )
    return in_maps


def _exact_dist_rows(rows, im_q, W1, b1, W2, b2, qs64, iD64):
    X = im_q[rows].astype(np.float32)
    h = np.maximum(
        (X @ W1.astype(np.float32)).astype(np.float64) + b1.astype(np.float64), 0
    )
    q = h @ W2.astype(np.float64) + b2.astype(np.float64)
    q = q / np.maximum(np.linalg.norm(q, axis=1, keepdims=True), 1e-12)
    u = q @ iD64
    r = np.sum(u * q, axis=1)
    t = q @ (iD64 @ qs64.T)
    c2 = np.sum((qs64 @ iD64) * qs64, axis=1)
    quad = np.maximum(r[:, None] + c2[None, :] - 2 * t, 0)
    return np.sqrt(quad).mean(axis=1)


LAST_RESULTS = None
LAST_STATS = None


def gather_dist(res):
    """Device dist [P, 16] per core -> full [B] vector (float64)."""
    parts = []
    for i in range(NCORES):
        d = np.asarray(res.results[i]["dist"]).reshape(P, MH * NT)
        parts.append(d.T.reshape(-1))  # row t*128+p
    return np.concatenate(parts).astype(np.float64)


def kernel(im_q, output, sample_idx, W1, b1, W2, b2, queue, invD):
    global LAST_RESULTS, LAST_STATS
    from concourse.bass_utils import run_bass_kernel_spmd

    inp = {
        "im_q": im_q, "W1": W1, "b1": b1, "W2": W2, "b2": b2,
        "queue": queue, "invD": invD, "sample_idx": sample_idx,
    }
    im_q = np.ascontiguousarray(np.asarray(im_q, dtype=np.float32))
    output = np.asarray(output, dtype=np.float32)
    W1 = np.ascontiguousarray(np.asarray(W1, dtype=np.float32))
    b1 = np.asarray(b1, dtype=np.float32)
    W2 = np.ascontiguousarray(np.asarray(W2, dtype=np.float32))
    b2 = np.asarray(b2, dtype=np.float32)
    queue = np.asarray(queue, dtype=np.float32)
    invD = np.ascontiguousarray(np.asarray(invD, dtype=np.float32))
    sample_idx = np.asarray(sample_idx)

    in_maps = per_core_inputs(inp)
    nc = _build_nc()
    res = run_bass_kernel_spmd(nc, in_maps, core_ids=list(range(NCORES)))
    LAST_RESULTS = res
    dist = gather_dist(res)

    # exact host recompute of rows near the top-64 inclusion boundary (and
    # the max-exclusion boundary) so device error cannot flip the set
    qs64 = queue[:, sample_idx].T.astype(np.float64)
    iD64 = invD.astype(np.float64)
    win = WINDOW
    done = np.zeros(B, dtype=bool)
    max_err = 0.0
    for _attempt in range(5):
        thr = np.partition(dist, B - NUM)[B - NUM]
        top1 = dist.max()
        rows = np.nonzero(
            ((np.abs(dist - thr) <= win) | (dist >= top1 - win)) & ~done
        )[0]
        if rows.size:
            prev = dist[rows].copy()
            dist[rows] = _exact_dist_rows(rows, im_q, W1, b1, W2, b2, qs64, iD64)
            max_err = max(max_err, float(np.abs(dist[rows] - prev).max()))
            done[rows] = True
        thr = np.partition(dist, B - NUM)[B - NUM]
        top1 = dist.max()
        chk = np.nonzero(
            ((np.abs(dist - thr) <= win / 2) | (dist >= top1 - win / 2)) & ~done
        )[0]
        if chk.size == 0:
            break
        win *= 1.6

    LAST_STATS = {
        "recompute_rows": int(done.sum()),
        "max_dev_err_at_boundary": max_err,
        "window": win,
    }
    order = np.argsort(dist, kind="stable")
    sel = order[-NUM:-1]
    row_mask = np.zeros(B, dtype=bool)
    row_mask[sel] = True
    cond = row_mask & ((np.abs(output[:, 2]) < 1.0) | (np.abs(output[:, 3]) < 1.0))
    out = output.copy()
    out[:, 2] = np.where(cond, np.float32(-5.0), output[:, 2])
    out[:, 3] = np.where(cond, np.float32(5.0), out[:, 3])
    return out


# revision 3
# speedup vs baseline: 1605.6530x; 1.0409x over previous
"""Trainium2 Bass kernel for nn_MoCo_4810363372846 (retrieval_knn) — v2.

Computation (see harness reference):
    h    = relu(im_q @ W1 + b1)            [B, 2048]
    q    = (h @ W2 + b2) row-normalized    [B, 128]
    dist = mean_j sqrt((q_i-k_j) invD (q_i-k_j)^T)  over 64 sampled queue cols
    top-63 (excluding the max) rows of dist gate a masked write into
    output[:, 2:4].

Strategy (v2):
  * Data-parallel over the B=16384 rows: 8 NeuronCores x 2048 rows each.
  * fp8 DoubleRow GEMMs for both layers (4x bf16 PE rate); hidden
    activations e4m3.
  * Row-major Mahalanobis tail: for each 128-row tile, the PE (with q as
    the bf16 stationary operand) emits q^T (plain matmul against the bf16
    identity — NOT is_transpose, whose PE mode switch costs >1us each on
    real hw), z=q^T invD and T=q^T (invD K); DVE scalar_tensor_tensor
    with accum_out fuses the row reductions (|q|^2, q^T invD q) and the
    u*T - c2/2 assembly; a single ACT Sqrt with per-partition bias +
    free-dim accumulator emits dist for 128 rows.  Normalization is
    folded into per-row scalars (no explicit normalize of q).
  * invD / invD K / q in bf16 (error absorbed by the host-side exact
    recompute of rows near the top-64 boundary); chain steps are woven
    between GEMM1 groups so the in-order PE never idles on them.
  * DMA order feeds the PE critical path first (w1n0, x8 m0 in pieces,
    rest of w1, GEMM2/chain constants, x8 m1 pieces, x8 m2-3); the x8 m1
    wait is filled with eagerly-emitted m0-chain work and partial m1
    groups.
  * Device output: dist [128, 16] per core (dist[p,t] = row t*128+p).
"""

import functools
import os

import numpy as np

B, DIM_MLP, DIM, KQ, NUM = 16384, 2048, 128, 16384, 64
NCORES = 8
P = 128
BL = B // NCORES        # 2048 rows per core
NH = 512                # rows per m-half (PSUM bank of fp32)
MH = BL // NH           # 4 m-halves
NT = NH // P            # 4 row-tiles per m-half
K16 = DIM_MLP // P      # 16 contraction sub-tiles
KA = 8                  # A-phase covers w1 n-tiles [0, KA)
SW = 8192.0             # host-side W1/W2 quantization scale
Q64 = 4096.0            # = NUM^2; folded so ACT accum emits mean directly

# dist window (absolute units) around the top-64 / top-1 thresholds whose
# rows get an exact host-side recompute; ~2.5x the observed max device err.
WINDOW = float(os.environ.get("KERNEL_WINDOW", "0.06"))

# timing-ablation knob (dev only; breaks numerics): "", "nochain", "notrans",
# "nostt"
ABLATE = os.environ.get("KERNEL_ABLATE", "")


@functools.lru_cache(maxsize=None)
def _build_nc(reps=1, hw_loop=False):
    import concourse.mybir as mybir
    import concourse.tile as tile
    from concourse import bacc

    f32 = mybir.dt.float32
    f32r = mybir.dt.float32r
    bf16 = mybir.dt.bfloat16
    f8 = mybir.dt.float8e4
    u8 = mybir.dt.uint8
    AF = mybir.ActivationFunctionType
    ALU = mybir.AluOpType
    DR = mybir.MatmulPerfMode.DoubleRow

    nc = bacc.Bacc(None, target_bir_lowering=False)

    x8 = nc.declare_dram_parameter("x8", [P, K16, BL], u8, isOutput=False)
    w1 = nc.declare_dram_parameter("w1", [K16, P, K16, P], u8, isOutput=False)
    w2q = nc.declare_dram_parameter("w2q", [P, K16, P], u8, isOutput=False)
    b1t = nc.declare_dram_parameter("b1t", [P, K16], f32, isOutput=False)
    b2t = nc.declare_dram_parameter("b2t", [P, 1], f32, isOutput=False)
    invd = nc.declare_dram_parameter("invd", [P, P], bf16, isOutput=False)
    ct = nc.declare_dram_parameter("ct", [P, NUM], bf16, isOutput=False)
    c2b = nc.declare_dram_parameter("c2b", [P, NUM], f32, isOutput=False)
    identt = nc.declare_dram_parameter("identt", [P, P], bf16, isOutput=False)
    dist = nc.declare_dram_parameter("dist", [P, MH * NT], f32, isOutput=True)

    with tile.TileContext(nc) as tc:
        with (
            tc.tile_pool(name="const", bufs=1) as constp,
            tc.tile_pool(name="w1p", bufs=2) as w1p,
            tc.tile_pool(name="xin", bufs=2) as xinp,
            tc.tile_pool(name="ht", bufs=1) as htp,
            tc.tile_pool(name="qtp", bufs=2) as qtp,
            tc.tile_pool(name="sc", bufs=2) as scp,
            tc.tile_pool(name="rows", bufs=4) as rowsp,
            tc.tile_pool(name="ps_h", bufs=3, space="PSUM") as ps_h,
            tc.tile_pool(name="ps_q", bufs=2, space="PSUM") as ps_q,
            tc.tile_pool(name="ps_c", bufs=3, space="PSUM") as ps_c,
        ):
            b1s = constp.tile([P, K16], f32)
            b2s = constp.tile([P, 1], f32)
            invs = constp.tile([P, P], bf16)
            cts = constp.tile([P, NUM], bf16)
            c2s = constp.tile([P, NUM], f32)
            idents = constp.tile([P, P], bf16)
            w2s = constp.tile([P, K16, P], f8)
            dist_sb = constp.tile([P, MH * NT], f32)
            warm = constp.tile([P, 1], f32)

            # ---- deferred chain steps, woven between GEMM1 groups ----
            pending = []

            def emit_one():
                if pending:
                    pending.pop(0)()

            def chain_steps(m, htc, fine_tail=False):
                """Stage-paired bundles for m-half m: GEMM2 + qt, then per
                128-row tile: PE (transpose/z/T), DVE reductions + scalars,
                ACT sqrt-accumulate into a dist column."""
                qt = qtp.tile([P, NH], bf16, tag=f"qt{m % 2}", name="qt")
                sts = [dict() for _ in range(NT)]

                def sA(m=m, htc=htc, qt=qt):
                    pq = ps_q.tile([P, NH], f32, tag="pq")
                    for kk in range(K16 // 2):
                        nc.tensor.matmul(
                            pq,
                            w2s[:, 2 * kk : 2 * kk + 2, :],
                            htc[:, 2 * kk : 2 * kk + 2, :],
                            start=(kk == 0),
                            stop=(kk == K16 // 2 - 1),
                            perf_mode=DR,
                        )
                    nc.scalar.activation(
                        qt, pq, AF.Identity, bias=b2s[:, 0:1], scale=1.0 / SW
                    )

                def sG2(t, m=m, htc=htc, qt=qt):
                    # per-tile GEMM2 + qt slice (tail only)
                    pqt = ps_q.tile([P, NH], f32, tag="pq", name="pqt")
                    pq = pqt[:, 0:P]
                    for kk in range(K16 // 2):
                        nc.tensor.matmul(
                            pq,
                            w2s[:, 2 * kk : 2 * kk + 2, :],
                            htc[:, 2 * kk : 2 * kk + 2, t * P : (t + 1) * P],
                            start=(kk == 0),
                            stop=(kk == K16 // 2 - 1),
                            perf_mode=DR,
                        )
                    nc.scalar.activation(
                        qt[:, t * P : (t + 1) * P],
                        pq,
                        AF.Identity,
                        bias=b2s[:, 0:1],
                        scale=1.0 / SW,
                    )

                def sB(t, qt=qt, sts=sts):
                    # one PSUM bank holds z | T | qtT per tile.  qtT comes
                    # from a PLAIN matmul against the bf16 identity
                    # (qc^T @ I = q^T row-major) — mathematically a
                    # transpose but avoids the PE is_transpose mode switch.
                    st = sts[t]
                    qc = qt[:, t * P : (t + 1) * P]
                    cb = ps_c.tile([P, NH], f32, tag="cb", name="cb")
                    z = cb[:, 0:P]
                    T = cb[:, P : P + NUM]
                    qtT = cb[:, P + NUM : P + NUM + P]
                    nc.tensor.matmul(qtT, qc, idents)
                    nc.tensor.matmul(z, qc, invs)
                    nc.tensor.matmul(T, qc, cts)
                    st["qtT"], st["z"], st["T"] = qtT, z, T

                def sD(t, sts=sts):
                    st = sts[t]
                    qtT, z = st["qtT"], st["z"]
                    qtTs = scp.tile([P, P], bf16, tag="qtTs", name="qtTs")
                    nc.vector.tensor_copy(qtTs, qtT)
                    snb = scp.tile([P, P], bf16, tag="snb", name="snb")
                    n2 = rowsp.tile([P, 1], f32, tag="n2", name="n2")
                    szb = scp.tile([P, P], bf16, tag="szb", name="szb")
                    R = rowsp.tile([P, 1], f32, tag="R", name="R")
                    if ABLATE == "nostt":
                        AX = mybir.AxisListType
                        nc.vector.tensor_mul(snb, qtTs, qtTs)
                        nc.vector.tensor_reduce(n2, snb, AX.X, ALU.add)
                        nc.vector.tensor_mul(szb, z, qtTs)
                        nc.vector.tensor_reduce(R, szb, AX.X, ALU.add)
                    else:
                        nc.vector.scalar_tensor_tensor(
                            snb, qtTs, 1.0, qtTs, ALU.mult, ALU.mult, accum_out=n2
                        )
                        nc.vector.scalar_tensor_tensor(
                            szb, z, 1.0, qtTs, ALU.mult, ALU.mult, accum_out=R
                        )
                    w = rowsp.tile([P, 1], f32, tag="w", name="w")
                    nc.vector.reciprocal(w, n2)
                    Rw = rowsp.tile([P, 1], f32, tag="Rw", name="Rw")
                    nc.vector.tensor_scalar(Rw, R, w, 1.0 / Q64, ALU.mult, ALU.mult)
                    st["w"], st["Rw"] = w, Rw

                def sE(t, m=m, sts=sts):
                    st = sts[t]
                    u = rowsp.tile([P, 1], f32, tag="u", name="u")
                    nc.scalar.activation(u, st["w"], AF.Sqrt)
                    xp = scp.tile([P, NUM], f32, tag="xp", name="xp")
                    if ABLATE == "nostt":
                        nc.vector.tensor_sub(xp, st["T"], c2s)
                    else:
                        nc.vector.scalar_tensor_tensor(
                            xp, st["T"], u, c2s, ALU.mult, ALU.subtract
                        )
                    sqs = scp.tile([P, NUM], bf16, tag="sqs", name="sqs")
                    g = m * NT + t
                    nc.scalar.activation(
                        sqs,
                        xp,
                        AF.Sqrt,
                        bias=st["Rw"],
                        scale=-2.0 / Q64,
                        accum_out=dist_sb[:, g : g + 1],
                    )

                def sOut(m=m, t0=0, t1=NT):
                    nc.sync.dma_start(
                        dist[:, m * NT + t0 : m * NT + t1],
                        dist_sb[:, m * NT + t0 : m * NT + t1],
                    )

                if ABLATE == "nochain":
                    return [sA]
                if not fine_tail:
                    steps = [
                        sA,
                        lambda: sB(0),
                        lambda: sB(1),
                        lambda: sD(0),
                        lambda: (sB(2), sD(1)),
                        lambda: (sE(0), sB(3), sD(2)),
                        lambda: (sE(1), sD(3)),
                        lambda: sE(2),
                        lambda: (sE(3), sOut()),
                    ]
                else:
                    steps = [
                        lambda: sG2(0),
                        lambda: sG2(1),
                        lambda: sG2(2),
                        lambda: (sB(0), sD(0)),
                        lambda: sG2(3),
                        lambda: (sB(1), sD(1), sE(0)),
                        lambda: (sB(2), sD(2), sE(1), sOut(t0=0, t1=1)),
                        lambda: (sB(3), sD(3), sE(2), sOut(t0=1, t1=2)),
                        lambda: (sE(3), sOut(t0=2, t1=3), sOut(t0=3, t1=4)),
                    ]
                return steps

            def dma_w1(n):
                t = w1p.tile([P, K16, P], f8, tag=f"w1n{n}", name=f"w1n{n}")
                nc.sync.dma_start(t, w1[n].bitcast(f8))
                return t

            def dma_x8_m(m, pieces=1, between=None):
                """Returns (list of piece tiles, k-tiles per piece).  Pieces
                are separate tiles so a consumer of an early k-tile does not
                wait on later pieces (tile-granular dependencies).  `between`
                (piece-index -> fn) lets small DMAs slot into the stream."""
                o0 = m * NH
                kper = K16 // pieces
                ts = []
                for a in range(pieces):
                    t = xinp.tile(
                        [P, kper, NH], f8, tag=f"x8m{m}p{a}", name=f"x8m{m}p{a}"
                    )
                    nc.sync.dma_start(
                        t,
                        x8.bitcast(f8)[:, a * kper : (a + 1) * kper, o0 : o0 + NH],
                    )
                    ts.append(t)
                    if between and a in between:
                        between[a]()
                return ts, kper

            def dma_consts_g2():
                nc.sync.dma_start(b2s, b2t[:])
                nc.sync.dma_start(w2s, w2q[:].bitcast(f8))

            def dma_consts_chain():
                nc.sync.dma_start(invs, invd[:])
                nc.sync.dma_start(cts, ct[:])
                nc.sync.dma_start(c2s, c2b[:])
                nc.sync.dma_start(idents, identt[:])

            def dma_consts():
                dma_consts_g2()
                dma_consts_chain()

            def emit_group(m, n, w1t, x8m, htc):
                pieces, kper = x8m
                ph = ps_h.tile([P, NH], f32, tag="ph", name="ph")
                for kk in range(K16 // 2):
                    k0 = 2 * kk
                    pc = pieces[k0 // kper]
                    off = k0 % kper
                    nc.tensor.matmul(
                        ph,
                        w1t[n][:, k0 : k0 + 2, :],
                        pc[:, off : off + 2, :],
                        start=(kk == 0),
                        stop=(kk == K16 // 2 - 1),
                        perf_mode=DR,
                    )
                nc.scalar.activation(
                    htc[:, n, :], ph, AF.Relu, bias=b1s[:, n : n + 1], scale=1.0 / SW
                )
                emit_one()

            def emit_rep(w1t, xs, htcs):
                for m in range(MH):
                    for n in range(K16):
                        emit_group(m, n, w1t, xs[m], htcs[m])
                    steps = chain_steps(m, htcs[m], fine_tail=(m == MH - 1))
                    if m == 0:
                        # eager: GEMM2(m0) + first two tile transposes fill
                        # the PE's wait for x8 m1 (still in the DMA pipe)
                        for s in steps[:3]:
                            s()
                        pending.extend(steps[3:])
                    else:
                        pending.extend(steps)

            def alloc_htcs():
                return [
                    htp.tile([P, K16, NH], f8, tag=f"h{m}", name=f"h{m}")
                    for m in range(MH)
                ]

            # ---- head: force the sqrt act table while DMA streams ----
            nc.vector.memset(warm, 1.0)
            nc.scalar.activation(warm, warm, AF.Sqrt)

            if hw_loop:
                nc.sync.dma_start(b1s, b1t[:])
                dma_consts()
                with tc.For_i(0, reps, 1):
                    w1t = [dma_w1(n) for n in range(K16)]
                    xs = [dma_x8_m(m) for m in range(MH)]
                    emit_rep(w1t, xs, alloc_htcs())
                    while pending:
                        emit_one()
            else:
                for r in range(reps):
                    if r == 0:
                        # w1n0, x8 m0 pieces (b1 slotted mid-stream), the
                        # rest of w1, the GEMM2 + chain constants (needed by
                        # the eagerly-emitted m0 chain), then x8 m1 in
                        # pieces (partial m1 groups start as they land),
                        # x8 m2-3.  m0 is fed at stream rate and the x8 m1
                        # wait is filled with chain + partial-group work.
                        w1t = [dma_w1(0)]
                        xs = [
                            dma_x8_m(
                                0,
                                pieces=4,
                                between={1: lambda: nc.sync.dma_start(b1s, b1t[:])},
                            )
                        ]
                        w1t += [dma_w1(n) for n in range(1, K16)]
                        dma_consts_g2()
                        nc.sync.dma_start(invs, invd[:])
                        nc.sync.dma_start(cts, ct[:])
                        nc.sync.dma_start(idents, identt[:])
                        xs.append(dma_x8_m(1, pieces=4))
                        nc.sync.dma_start(c2s, c2b[:])
                        xs += [dma_x8_m(m) for m in range(2, MH)]
                    else:
                        w1t = [dma_w1(0)]
                        xs = [dma_x8_m(0)]
                        w1t += [dma_w1(n) for n in range(1, K16)]
                        xs += [dma_x8_m(m) for m in range(1, MH)]
                    emit_rep(w1t, xs, alloc_htcs())
                while pending:
                    emit_one()

    nc.compile()
    return nc


def _host_constants(W1, b1, W2, b2, queue, invD, sample_idx):
    import ml_dtypes

    E4 = ml_dtypes.float8_e4m3
    BF = ml_dtypes.bfloat16
    qs = queue[:, sample_idx].T.astype(np.float64)  # [64, 128]
    iD = invD.astype(np.float64)
    ct = (iD @ qs.T).astype(np.float32)  # [128, 64]
    c2 = np.sum((qs @ iD) * qs, axis=1).astype(np.float32)  # [64]
    b1t = np.ascontiguousarray(b1.astype(np.float32).reshape(K16, P).T)
    b2t = np.ascontiguousarray(b2.astype(np.float32).reshape(P, 1))
    c2b = np.ascontiguousarray(
        np.broadcast_to((c2 / 2.0)[None, :], (P, NUM)).astype(np.float32)
    )
    identt = np.eye(P, dtype=BF)
    w1q = np.ascontiguousarray(
        (W1 * np.float32(SW))
        .astype(E4)
        .reshape(K16, P, K16, P)
        .transpose(2, 1, 0, 3)
    ).view(np.uint8)
    w2q = np.ascontiguousarray(
        (W2 * np.float32(SW)).astype(E4).reshape(K16, P, DIM).transpose(1, 0, 2)
    ).view(np.uint8)
    return ct, c2, b1t, b2t, w1q, w2q, c2b, identt


def _host_x8(im_q):
    import ml_dtypes

    return np.ascontiguousarray(
        im_q.astype(ml_dtypes.float8_e4m3)
        .reshape(NCORES, BL, K16, P)
        .transpose(0, 3, 2, 1)
    ).view(np.uint8)


def per_core_inputs(inp):
    im_q = np.ascontiguousarray(np.asarray(inp["im_q"], dtype=np.float32))
    W1 = np.ascontiguousarray(np.asarray(inp["W1"], dtype=np.float32))
    b1 = np.asarray(inp["b1"], dtype=np.float32)
    W2 = np.ascontiguousarray(np.asarray(inp["W2"], dtype=np.float32))
    b2 = np.asarray(inp["b2"], dtype=np.float32)
    queue = np.asarray(inp["queue"], dtype=np.float32)
    invD = np.ascontiguousarray(np.asarray(inp["invD"], dtype=np.float32))
    sample_idx = np.asarray(inp["sample_idx"])

    ct, c2, b1t, b2t, w1q, w2q, c2b, identt = _host_constants(
        W1, b1, W2, b2, queue, invD, sample_idx
    )
    import ml_dtypes

    BF = ml_dtypes.bfloat16
    invb = np.ascontiguousarray(invD.astype(BF))
    ctb = np.ascontiguousarray(ct.astype(BF))
    x8 = _host_x8(im_q)
    in_maps = []
    for i in range(NCORES):
        in_maps.append(
            {
                "x8": x8[i],
                "w1": w1q,
                "w2q": w2q,
                "b1t": b1t,
                "b2t": b2t,
                "invd": invb,
                "ct": ctb,
                "c2b": c2b,
                "identt": identt,
            }
        )
    return in_maps


def _exact_dist_rows(rows, im_q, W1, b1, W2, b2, qs64, iD64):
    X = im_q[rows].astype(np.float32)
    h = np.maximum(
        (X @ W1.astype(np.float32)).astype(np.float64) + b1.astype(np.float64), 0
    )
    q = h @ W2.astype(np.float64) + b2.astype(np.float64)
    q = q / np.maximum(np.linalg.norm(q, axis=1, keepdims=True), 1e-12)
    u = q @ iD64
    r = np.sum(u * q, axis=1)
    t = q @ (iD64 @ qs64.T)
    c2 = np.sum((qs64 @ iD64) * qs64, axis=1)
    quad = np.maximum(r[:, None] + c2[None, :] - 2 * t, 0)
    return np.sqrt(quad).mean(axis=1)


LAST_RESULTS = None
LAST_STATS = None


def gather_dist(res):
    """Device dist [P, 16] per core -> full [B] vector (float64)."""
    parts = []
    for i in range(NCORES):
        d = np.asarray(res.results[i]["dist"]).reshape(P, MH * NT)
        parts.append(d.T.reshape(-1))  # row t*128+p
    return np.concatenate(parts).astype(np.float64)


def kernel(im_q, output, sample_idx, W1, b1, W2, b2, queue, invD):
    global LAST_RESULTS, LAST_STATS
    from concourse.bass_utils import run_bass_kernel_spmd

    inp = {
        "im_q": im_q, "W1": W1, "b1": b1, "W2": W2, "b2": b2,
        "queue": queue, "invD": invD, "sample_idx": sample_idx,
    }
    im_q = np.ascontiguousarray(np.asarray(im_q, dtype=np.float32))
    output = np.asarray(output, dtype=np.float32)
    W1 = np.ascontiguousarray(np.asarray(W1, dtype=np.float32))
    b1 = np.asarray(b1, dtype=np.float32)
    W2 = np.ascontiguousarray(np.asarray(W2, dtype=np.float32))
    b2 = np.asarray(b2, dtype=np.float32)
    queue = np.asarray(queue, dtype=np.float32)
    invD = np.ascontiguousarray(np.asarray(invD, dtype=np.float32))
    sample_idx = np.asarray(sample_idx)

    in_maps = per_core_inputs(inp)
    nc = _build_nc()
    res = run_bass_kernel_spmd(nc, in_maps, core_ids=list(range(NCORES)))
    LAST_RESULTS = res
    dist = gather_dist(res)

    # exact host recompute of rows near the top-64 inclusion boundary (and
    # the max-exclusion boundary) so device error cannot flip the set
    qs64 = queue[:, sample_idx].T.astype(np.float64)
    iD64 = invD.astype(np.float64)
    win = WINDOW
    done = np.zeros(B, dtype=bool)
    max_err = 0.0
    for _attempt in range(5):
        thr = np.partition(dist, B - NUM)[B - NUM]
        top1 = dist.max()
        rows = np.nonzero(
            ((np.abs(dist - thr) <= win) | (dist >= top1 - win)) & ~done
        )[0]
        if rows.size:
            prev = dist[rows].copy()
            dist[rows] = _exact_dist_rows(rows, im_q, W1, b1, W2, b2, qs64, iD64)
            max_err = max(max_err, float(np.abs(dist[rows] - prev).max()))
            done[rows] = True
        thr = np.partition(dist, B - NUM)[B - NUM]
        top1 = dist.max()
        chk = np.nonzero(
            ((np.abs(dist - thr) <= win / 2) | (dist >= top1 - win / 2)) & ~done
        )[0]
        if chk.size == 0:
            break
        win *= 1.6

    LAST_STATS = {
        "recompute_rows": int(done.sum()),
        "max_dev_err_at_boundary": max_err,
        "window": win,
    }
    order = np.argsort(dist, kind="stable")
    sel = order[-NUM:-1]
    row_mask = np.zeros(B, dtype=bool)
    row_mask[sel] = True
    cond = row_mask & ((np.abs(output[:, 2]) < 1.0) | (np.abs(output[:, 3]) < 1.0))
    out = output.copy()
    out[:, 2] = np.where(cond, np.float32(-5.0), output[:, 2])
    out[:, 3] = np.where(cond, np.float32(5.0), out[:, 3])
    return out


# revision 4
# speedup vs baseline: 2070.0433x; 1.2892x over previous
"""Trainium2 Bass kernel for nn_MoCo_4810363372846 (retrieval_knn) — v2.

Computation (see harness reference):
    h    = relu(im_q @ W1 + b1)            [B, 2048]
    q    = (h @ W2 + b2) row-normalized    [B, 128]
    dist = mean_j sqrt((q_i-k_j) invD (q_i-k_j)^T)  over 64 sampled queue cols
    top-63 (excluding the max) rows of dist gate a masked write into
    output[:, 2:4].

Strategy (v2):
  * Data-parallel over the B=16384 rows: 8 NeuronCores x 2048 rows each.
  * fp8 DoubleRow GEMMs for both layers (4x bf16 PE rate); hidden
    activations e4m3.
  * Row-major Mahalanobis tail: for each 128-row tile, the PE (with q as
    the bf16 stationary operand) emits q^T (plain matmul against the bf16
    identity — NOT is_transpose, whose PE mode switch costs >1us each on
    real hw), z=q^T invD and T=q^T (invD K); DVE scalar_tensor_tensor
    with accum_out fuses the row reductions (|q|^2, q^T invD q) and the
    u*T - c2/2 assembly; a single ACT Sqrt with per-partition bias +
    free-dim accumulator emits dist for 128 rows.  Normalization is
    folded into per-row scalars (no explicit normalize of q).
  * invD / invD K / q in bf16 (error absorbed by the host-side exact
    recompute of rows near the top-64 boundary); chain steps are woven
    between GEMM1 groups so the in-order PE never idles on them.
  * DMA order feeds the PE critical path first (w1n0, x8 m0 in pieces,
    rest of w1, GEMM2/chain constants, x8 m1 pieces, x8 m2-3); the x8 m1
    wait is filled with eagerly-emitted m0-chain work and partial m1
    groups.
  * Device output: dist [128, 16] per core (dist[p,t] = row t*128+p).
"""

import functools
import os

import numpy as np

B, DIM_MLP, DIM, KQ, NUM = 16384, 2048, 128, 16384, 64
NCORES = 8
P = 128
BL = B // NCORES        # 2048 rows per core
NH = 512                # rows per m-half (PSUM bank of fp32)
MH = BL // NH           # 4 m-halves
NT = NH // P            # 4 row-tiles per m-half
K16 = DIM_MLP // P      # 16 contraction sub-tiles
KA = 8                  # A-phase covers w1 n-tiles [0, KA)
SW = 8192.0             # host-side W1/W2 quantization scale
Q64 = 4096.0            # = NUM^2; folded so ACT accum emits mean directly

# dist window (absolute units) around the top-64 / top-1 thresholds whose
# rows get an exact host-side recompute; ~2.5x the observed max device err.
WINDOW = float(os.environ.get("KERNEL_WINDOW", "0.06"))

# timing-ablation knob (dev only; breaks numerics): "", "nochain", "notrans",
# "nostt"
ABLATE = os.environ.get("KERNEL_ABLATE", "")


@functools.lru_cache(maxsize=None)
def _build_nc(reps=1, hw_loop=False):
    import concourse.mybir as mybir
    import concourse.tile as tile
    from concourse import bacc

    f32 = mybir.dt.float32
    f32r = mybir.dt.float32r
    bf16 = mybir.dt.bfloat16
    f8 = mybir.dt.float8e4
    u8 = mybir.dt.uint8
    AF = mybir.ActivationFunctionType
    ALU = mybir.AluOpType
    DR = mybir.MatmulPerfMode.DoubleRow

    nc = bacc.Bacc(None, target_bir_lowering=False)

    x8 = nc.declare_dram_parameter("x8", [P, K16, BL], u8, isOutput=False)
    w1 = nc.declare_dram_parameter("w1", [K16, P, K16, P], u8, isOutput=False)
    w2q = nc.declare_dram_parameter("w2q", [P, K16, P], u8, isOutput=False)
    b1t = nc.declare_dram_parameter("b1t", [P, K16], f32, isOutput=False)
    b2t = nc.declare_dram_parameter("b2t", [P, 1], f32, isOutput=False)
    chw = nc.declare_dram_parameter(
        "chw", [P, 2 * P + NUM], bf16, isOutput=False
    )  # [I | invD | invD K] concatenated chain weights
    c2b = nc.declare_dram_parameter("c2b", [P, NUM], f32, isOutput=False)
    dist = nc.declare_dram_parameter("dist", [P, MH * NT], f32, isOutput=True)

    with tile.TileContext(nc) as tc:
        with (
            tc.tile_pool(name="const", bufs=1) as constp,
            tc.tile_pool(name="w1p", bufs=2) as w1p,
            tc.tile_pool(name="xin", bufs=2) as xinp,
            tc.tile_pool(name="ht", bufs=1) as htp,
            tc.tile_pool(name="qtp", bufs=2) as qtp,
            tc.tile_pool(name="sc", bufs=2) as scp,
            tc.tile_pool(name="rows", bufs=4) as rowsp,
            tc.tile_pool(name="ps_h", bufs=3, space="PSUM") as ps_h,
            tc.tile_pool(name="ps_q", bufs=2, space="PSUM") as ps_q,
            tc.tile_pool(name="ps_c", bufs=3, space="PSUM") as ps_c,
        ):
            b1s = constp.tile([P, K16], f32)
            b2s = constp.tile([P, 1], f32)
            chws = constp.tile([P, 2 * P + NUM], bf16)
            c2s = constp.tile([P, NUM], f32)
            w2s = constp.tile([P, K16, P], f8)
            dist_sb = constp.tile([P, MH * NT], f32)
            warm = constp.tile([P, 1], f32)

            # ---- deferred chain steps, woven between GEMM1 groups ----
            pending = []

            def emit_one():
                if pending:
                    pending.pop(0)()

            def chain_steps(m, htc, fine_tail=False):
                """Stage-paired bundles for m-half m: GEMM2 + qt, then per
                128-row tile: PE (transpose/z/T), DVE reductions + scalars,
                ACT sqrt-accumulate into a dist column."""
                qt = qtp.tile([P, NH], bf16, tag=f"qt{m % 2}", name="qt")
                sts = [dict() for _ in range(NT)]

                def sA(m=m, htc=htc, qt=qt):
                    pq = ps_q.tile([P, NH], f32, tag="pq")
                    for kk in range(K16 // 2):
                        nc.tensor.matmul(
                            pq,
                            w2s[:, 2 * kk : 2 * kk + 2, :],
                            htc[:, 2 * kk : 2 * kk + 2, :],
                            start=(kk == 0),
                            stop=(kk == K16 // 2 - 1),
                            perf_mode=DR,
                        )
                    nc.scalar.activation(
                        qt, pq, AF.Identity, bias=b2s[:, 0:1], scale=1.0 / SW
                    )

                def sG2(t, m=m, htc=htc, qt=qt):
                    # per-tile GEMM2 + qt slice (tail only)
                    pqt = ps_q.tile([P, NH], f32, tag="pq", name="pqt")
                    pq = pqt[:, 0:P]
                    for kk in range(K16 // 2):
                        nc.tensor.matmul(
                            pq,
                            w2s[:, 2 * kk : 2 * kk + 2, :],
                            htc[:, 2 * kk : 2 * kk + 2, t * P : (t + 1) * P],
                            start=(kk == 0),
                            stop=(kk == K16 // 2 - 1),
                            perf_mode=DR,
                        )
                    nc.scalar.activation(
                        qt[:, t * P : (t + 1) * P],
                        pq,
                        AF.Identity,
                        bias=b2s[:, 0:1],
                        scale=1.0 / SW,
                    )

                def sB(t, qt=qt, sts=sts):
                    # ONE matmul per tile: qc^T @ [I | invD | invD K] lands
                    # q^T (row-major transpose, WITHOUT the PE is_transpose
                    # mode switch, which costs >1us each on real hw), z and
                    # T side by side in a single PSUM bank.
                    st = sts[t]
                    qc = qt[:, t * P : (t + 1) * P]
                    cb = ps_c.tile([P, NH], f32, tag="cb", name="cb")
                    nc.tensor.matmul(cb[:, 0 : 2 * P + NUM], qc, chws)
                    st["qtT"] = cb[:, 0:P]
                    st["z"] = cb[:, P : 2 * P]
                    st["T"] = cb[:, 2 * P : 2 * P + NUM]

                def sD(t, sts=sts):
                    st = sts[t]
                    qtT, z = st["qtT"], st["z"]
                    qtTs = scp.tile([P, P], bf16, tag="qtTs", name="qtTs")
                    nc.vector.tensor_copy(qtTs, qtT)
                    snb = scp.tile([P, P], bf16, tag="snb", name="snb")
                    n2 = rowsp.tile([P, 1], f32, tag="n2", name="n2")
                    szb = scp.tile([P, P], bf16, tag="szb", name="szb")
                    R = rowsp.tile([P, 1], f32, tag="R", name="R")
                    if ABLATE == "nostt":
                        AX = mybir.AxisListType
                        nc.vector.tensor_mul(snb, qtTs, qtTs)
                        nc.vector.tensor_reduce(n2, snb, AX.X, ALU.add)
                        nc.vector.tensor_mul(szb, z, qtTs)
                        nc.vector.tensor_reduce(R, szb, AX.X, ALU.add)
                    else:
                        nc.vector.scalar_tensor_tensor(
                            snb, qtTs, 1.0, qtTs, ALU.mult, ALU.mult, accum_out=n2
                        )
                        nc.vector.scalar_tensor_tensor(
                            szb, z, 1.0, qtTs, ALU.mult, ALU.mult, accum_out=R
                        )
                    w = rowsp.tile([P, 1], f32, tag="w", name="w")
                    nc.vector.reciprocal(w, n2)
                    Rw = rowsp.tile([P, 1], f32, tag="Rw", name="Rw")
                    nc.vector.tensor_scalar(Rw, R, w, 1.0 / Q64, ALU.mult, ALU.mult)
                    st["w"], st["Rw"] = w, Rw

                def sE(t, m=m, sts=sts):
                    st = sts[t]
                    u = rowsp.tile([P, 1], f32, tag="u", name="u")
                    nc.scalar.activation(u, st["w"], AF.Sqrt)
                    xp = scp.tile([P, NUM], f32, tag="xp", name="xp")
                    if ABLATE == "nostt":
                        nc.vector.tensor_sub(xp, st["T"], c2s)
                    else:
                        nc.vector.scalar_tensor_tensor(
                            xp, st["T"], u, c2s, ALU.mult, ALU.subtract
                        )
                    sqs = scp.tile([P, NUM], bf16, tag="sqs", name="sqs")
                    g = m * NT + t
                    nc.scalar.activation(
                        sqs,
                        xp,
                        AF.Sqrt,
                        bias=st["Rw"],
                        scale=-2.0 / Q64,
                        accum_out=dist_sb[:, g : g + 1],
                    )

                def sOut(m=m, t0=0, t1=NT):
                    nc.sync.dma_start(
                        dist[:, m * NT + t0 : m * NT + t1],
                        dist_sb[:, m * NT + t0 : m * NT + t1],
                    )

                if ABLATE == "nochain":
                    return [sA]
                if not fine_tail:
                    steps = [
                        sA,
                        lambda: sB(0),
                        lambda: sB(1),
                        lambda: sD(0),
                        lambda: (sB(2), sD(1)),
                        lambda: (sE(0), sB(3), sD(2)),
                        lambda: (sE(1), sD(3)),
                        lambda: sE(2),
                        lambda: (sE(3), sOut()),
                    ]
                else:
                    steps = [
                        lambda: sG2(0),
                        lambda: sG2(1),
                        lambda: sG2(2),
                        lambda: (sB(0), sD(0)),
                        lambda: sG2(3),
                        lambda: (sB(1), sD(1), sE(0)),
                        lambda: (sB(2), sD(2), sE(1), sOut(t0=0, t1=1)),
                        lambda: (sB(3), sD(3), sE(2), sOut(t0=1, t1=2)),
                        lambda: (sE(3), sOut(t0=2, t1=3), sOut(t0=3, t1=4)),
                    ]
                return steps

            def dma_w1(n):
                t = w1p.tile([P, K16, P], f8, tag=f"w1n{n}", name=f"w1n{n}")
                nc.sync.dma_start(t, w1[n].bitcast(f8))
                return t

            def dma_x8_m(m, pieces=1, between=None):
                """Returns (list of piece tiles, k-tiles per piece).  Pieces
                are separate tiles so a consumer of an early k-tile does not
                wait on later pieces (tile-granular dependencies).  `between`
                (piece-index -> fn) lets small DMAs slot into the stream."""
                o0 = m * NH
                kper = K16 // pieces
                ts = []
                for a in range(pieces):
                    t = xinp.tile(
                        [P, kper, NH], f8, tag=f"x8m{m}p{a}", name=f"x8m{m}p{a}"
                    )
                    nc.sync.dma_start(
                        t,
                        x8.bitcast(f8)[:, a * kper : (a + 1) * kper, o0 : o0 + NH],
                    )
                    ts.append(t)
                    if between and a in between:
                        between[a]()
                return ts, kper

            def dma_consts_g2():
                nc.sync.dma_start(b2s, b2t[:])
                nc.sync.dma_start(w2s, w2q[:].bitcast(f8))

            def dma_consts_chain():
                nc.sync.dma_start(chws, chw[:])
                nc.sync.dma_start(c2s, c2b[:])

            def dma_consts():
                dma_consts_g2()
                dma_consts_chain()

            def emit_group(m, n, w1t, x8m, htc):
                pieces, kper = x8m
                ph = ps_h.tile([P, NH], f32, tag="ph", name="ph")
                for kk in range(K16 // 2):
                    k0 = 2 * kk
                    pc = pieces[k0 // kper]
                    off = k0 % kper
                    nc.tensor.matmul(
                        ph,
                        w1t[n][:, k0 : k0 + 2, :],
                        pc[:, off : off + 2, :],
                        start=(kk == 0),
                        stop=(kk == K16 // 2 - 1),
                        perf_mode=DR,
                    )
                nc.scalar.activation(
                    htc[:, n, :], ph, AF.Relu, bias=b1s[:, n : n + 1], scale=1.0 / SW
                )
                emit_one()

            def emit_rep(w1t, xs, htcs):
                for m in range(MH):
                    for n in range(K16):
                        emit_group(m, n, w1t, xs[m], htcs[m])
                    steps = chain_steps(m, htcs[m], fine_tail=(m == MH - 1))
                    if m == 0:
                        # eager: GEMM2(m0) + first two tile transposes fill
                        # the PE's wait for x8 m1 (still in the DMA pipe)
                        for s in steps[:3]:
                            s()
                        pending.extend(steps[3:])
                    else:
                        pending.extend(steps)

            def alloc_htcs():
                return [
                    htp.tile([P, K16, NH], f8, tag=f"h{m}", name=f"h{m}")
                    for m in range(MH)
                ]

            # ---- head: force the sqrt act table while DMA streams ----
            nc.vector.memset(warm, 1.0)
            nc.scalar.activation(warm, warm, AF.Sqrt)

            if hw_loop:
                nc.sync.dma_start(b1s, b1t[:])
                dma_consts()
                with tc.For_i(0, reps, 1):
                    w1t = [dma_w1(n) for n in range(K16)]
                    xs = [dma_x8_m(m) for m in range(MH)]
                    emit_rep(w1t, xs, alloc_htcs())
                    while pending:
                        emit_one()
            else:
                for r in range(reps):
                    if r == 0:
                        # w1n0, x8 m0 pieces (b1 slotted mid-stream), the
                        # rest of w1, the GEMM2 + chain constants (needed by
                        # the eagerly-emitted m0 chain), then x8 m1 in
                        # pieces (partial m1 groups start as they land),
                        # x8 m2-3.  m0 is fed at stream rate and the x8 m1
                        # wait is filled with chain + partial-group work.
                        w1t = [dma_w1(0)]
                        xs = [
                            dma_x8_m(
                                0,
                                pieces=4,
                                between={1: lambda: nc.sync.dma_start(b1s, b1t[:])},
                            )
                        ]
                        w1t += [dma_w1(n) for n in range(1, K16)]
                        dma_consts_g2()
                        nc.sync.dma_start(chws, chw[:])
                        xs.append(dma_x8_m(1, pieces=4))
                        nc.sync.dma_start(c2s, c2b[:])
                        xs += [dma_x8_m(m) for m in range(2, MH)]
                    else:
                        w1t = [dma_w1(0)]
                        xs = [dma_x8_m(0)]
                        w1t += [dma_w1(n) for n in range(1, K16)]
                        xs += [dma_x8_m(m) for m in range(1, MH)]
                    emit_rep(w1t, xs, alloc_htcs())
                while pending:
                    emit_one()

    nc.compile()
    return nc


def _host_constants(W1, b1, W2, b2, queue, invD, sample_idx):
    import ml_dtypes

    E4 = ml_dtypes.float8_e4m3
    BF = ml_dtypes.bfloat16
    qs = queue[:, sample_idx].T.astype(np.float64)  # [64, 128]
    iD = invD.astype(np.float64)
    ct = (iD @ qs.T).astype(np.float32)  # [128, 64]
    c2 = np.sum((qs @ iD) * qs, axis=1).astype(np.float32)  # [64]
    b1t = np.ascontiguousarray(b1.astype(np.float32).reshape(K16, P).T)
    b2t = np.ascontiguousarray(b2.astype(np.float32).reshape(P, 1))
    c2b = np.ascontiguousarray(
        np.broadcast_to((c2 / 2.0)[None, :], (P, NUM)).astype(np.float32)
    )
    chw = np.ascontiguousarray(
        np.concatenate(
            [np.eye(P, dtype=np.float32), invD.astype(np.float32), ct], axis=1
        ).astype(BF)
    )
    w1q = np.ascontiguousarray(
        (W1 * np.float32(SW))
        .astype(E4)
        .reshape(K16, P, K16, P)
        .transpose(2, 1, 0, 3)
    ).view(np.uint8)
    w2q = np.ascontiguousarray(
        (W2 * np.float32(SW)).astype(E4).reshape(K16, P, DIM).transpose(1, 0, 2)
    ).view(np.uint8)
    return ct, c2, b1t, b2t, w1q, w2q, c2b, chw


def _host_x8(im_q):
    import ml_dtypes

    return np.ascontiguousarray(
        im_q.astype(ml_dtypes.float8_e4m3)
        .reshape(NCORES, BL, K16, P)
        .transpose(0, 3, 2, 1)
    ).view(np.uint8)


def per_core_inputs(inp):
    im_q = np.ascontiguousarray(np.asarray(inp["im_q"], dtype=np.float32))
    W1 = np.ascontiguousarray(np.asarray(inp["W1"], dtype=np.float32))
    b1 = np.asarray(inp["b1"], dtype=np.float32)
    W2 = np.ascontiguousarray(np.asarray(inp["W2"], dtype=np.float32))
    b2 = np.asarray(inp["b2"], dtype=np.float32)
    queue = np.asarray(inp["queue"], dtype=np.float32)
    invD = np.ascontiguousarray(np.asarray(inp["invD"], dtype=np.float32))
    sample_idx = np.asarray(inp["sample_idx"])

    ct, c2, b1t, b2t, w1q, w2q, c2b, chw = _host_constants(
        W1, b1, W2, b2, queue, invD, sample_idx
    )
    x8 = _host_x8(im_q)
    in_maps = []
    for i in range(NCORES):
        in_maps.append(
            {
                "x8": x8[i],
                "w1": w1q,
                "w2q": w2q,
                "b1t": b1t,
                "b2t": b2t,
                "chw": chw,
                "c2b": c2b,
            }
        )
    return in_maps


def _exact_dist_rows(rows, im_q, W1, b1, W2, b2, qs64, iD64):
    X = im_q[rows].astype(np.float32)
    h = np.maximum(
        (X @ W1.astype(np.float32)).astype(np.float64) + b1.astype(np.float64), 0
    )
    q = h @ W2.astype(np.float64) + b2.astype(np.float64)
    q = q / np.maximum(np.linalg.norm(q, axis=1, keepdims=True), 1e-12)
    u = q @ iD64
    r = np.sum(u * q, axis=1)
    t = q @ (iD64 @ qs64.T)
    c2 = np.sum((qs64 @ iD64) * qs64, axis=1)
    quad = np.maximum(r[:, None] + c2[None, :] - 2 * t, 0)
    return np.sqrt(quad).mean(axis=1)


LAST_RESULTS = None
LAST_STATS = None


def gather_dist(res):
    """Device dist [P, 16] per core -> full [B] vector (float64)."""
    parts = []
    for i in range(NCORES):
        d = np.asarray(res.results[i]["dist"]).reshape(P, MH * NT)
        parts.append(d.T.reshape(-1))  # row t*128+p
    return np.concatenate(parts).astype(np.float64)


def kernel(im_q, output, sample_idx, W1, b1, W2, b2, queue, invD):
    global LAST_RESULTS, LAST_STATS
    from concourse.bass_utils import run_bass_kernel_spmd

    inp = {
        "im_q": im_q, "W1": W1, "b1": b1, "W2": W2, "b2": b2,
        "queue": queue, "invD": invD, "sample_idx": sample_idx,
    }
    im_q = np.ascontiguousarray(np.asarray(im_q, dtype=np.float32))
    output = np.asarray(output, dtype=np.float32)
    W1 = np.ascontiguousarray(np.asarray(W1, dtype=np.float32))
    b1 = np.asarray(b1, dtype=np.float32)
    W2 = np.ascontiguousarray(np.asarray(W2, dtype=np.float32))
    b2 = np.asarray(b2, dtype=np.float32)
    queue = np.asarray(queue, dtype=np.float32)
    invD = np.ascontiguousarray(np.asarray(invD, dtype=np.float32))
    sample_idx = np.asarray(sample_idx)

    in_maps = per_core_inputs(inp)
    nc = _build_nc()
    res = run_bass_kernel_spmd(nc, in_maps, core_ids=list(range(NCORES)))
    LAST_RESULTS = res
    dist = gather_dist(res)

    # exact host recompute of rows near the top-64 inclusion boundary (and
    # the max-exclusion boundary) so device error cannot flip the set
    qs64 = queue[:, sample_idx].T.astype(np.float64)
    iD64 = invD.astype(np.float64)
    win = WINDOW
    done = np.zeros(B, dtype=bool)
    max_err = 0.0
    for _attempt in range(5):
        thr = np.partition(dist, B - NUM)[B - NUM]
        top1 = dist.max()
        rows = np.nonzero(
            ((np.abs(dist - thr) <= win) | (dist >= top1 - win)) & ~done
        )[0]
        if rows.size:
            prev = dist[rows].copy()
            dist[rows] = _exact_dist_rows(rows, im_q, W1, b1, W2, b2, qs64, iD64)
            max_err = max(max_err, float(np.abs(dist[rows] - prev).max()))
            done[rows] = True
        thr = np.partition(dist, B - NUM)[B - NUM]
        top1 = dist.max()
        chk = np.nonzero(
            ((np.abs(dist - thr) <= win / 2) | (dist >= top1 - win / 2)) & ~done
        )[0]
        if chk.size == 0:
            break
        win *= 1.6

    LAST_STATS = {
        "recompute_rows": int(done.sum()),
        "max_dev_err_at_boundary": max_err,
        "window": win,
    }
    order = np.argsort(dist, kind="stable")
    sel = order[-NUM:-1]
    row_mask = np.zeros(B, dtype=bool)
    row_mask[sel] = True
    cond = row_mask & ((np.abs(output[:, 2]) < 1.0) | (np.abs(output[:, 3]) < 1.0))
    out = output.copy()
    out[:, 2] = np.where(cond, np.float32(-5.0), output[:, 2])
    out[:, 3] = np.where(cond, np.float32(5.0), out[:, 3])
    return out
